# revision 6
# baseline (speedup 1.0000x reference)
"""BAM spatial self-attention on 8 TRN2 cores (data-parallel over batch).

v3: fp8 DoubleRow out-matmul. Per chunk: energy+exp (bf16) -> softmax
denominator S via ones-matmul -> quantize attn weights to e4m3 as
240*exp/S (bounded to (0,240] by construction) -> DoubleRow fp8
matmuls against fp8 v (2 key-blocks per MM). Scalar engine becomes the
bottleneck (pure exp stream); out-matmuls of chunk c overlap the
exp phase of chunk c+1.
"""
import sys
import numpy as np

for p in ("/opt/trn_rl_repo",):
    if p not in sys.path:
        sys.path.insert(0, p)

B, C, H, W = 8, 256, 64, 64
N = H * W          # 4096
CK = C // 8        # 32
NB = N // 128      # 32 key blocks
MC = N // 512      # 8 query chunks
NG = NB // 4       # 8 groups of 4 key blocks

_NC_CACHE = {}


def _build_nc():
    import concourse.mybir as mybir
    import concourse.tile as tile
    from concourse import bacc
    from concourse.bass import ds

    f32, f32r, bf16 = mybir.dt.float32, mybir.dt.float32r, mybir.dt.bfloat16
    fp8 = mybir.dt.float8e4
    Exp = mybir.ActivationFunctionType.Exp
    Identity = mybir.ActivationFunctionType.Identity
    DR = mybir.MatmulPerfMode.DoubleRow
    Mult = mybir.AluOpType.mult
    Add = mybir.AluOpType.add

    nc = bacc.Bacc("TRN2", target_bir_lowering=False, debug=False)

    x_d = nc.dram_tensor("x", [C, N], f32, kind="ExternalInput").ap()
    wq_d = nc.dram_tensor("Wq", [CK, C], f32, kind="ExternalInput").ap()
    bq_d = nc.dram_tensor("bq", [CK], f32, kind="ExternalInput").ap()
    wk_d = nc.dram_tensor("Wk", [CK, C], f32, kind="ExternalInput").ap()
    bk_d = nc.dram_tensor("bk", [CK], f32, kind="ExternalInput").ap()
    wv_d = nc.dram_tensor("Wv", [C, C], f32, kind="ExternalInput").ap()
    bv_d = nc.dram_tensor("bv", [C], f32, kind="ExternalInput").ap()
    g_d = nc.dram_tensor("gamma", [1], f32, kind="ExternalInput").ap()
    y_d = nc.dram_tensor("y", [C, N], f32, kind="ExternalOutput").ap()

    x_r = x_d.rearrange("(o p) n -> p o n", p=128)   # c = o*128 + p
    y_r = y_d.rearrange("(o p) n -> p o n", p=128)

    with tile.TileContext(nc) as tc:
        with tc.tile_pool(name="const", bufs=1) as const, \
             tc.tile_pool(name="big", bufs=1) as big, \
             tc.tile_pool(name="ptp", bufs=2) as ptp, \
             tc.tile_pool(name="pt8p", bufs=1) as pt8p, \
             tc.tile_pool(name="work", bufs=4) as work, \
             tc.tile_pool(name="ps_st", bufs=1, space="PSUM") as ps_st, \
             tc.tile_pool(name="ps_out", bufs=2, space="PSUM") as ps_out, \
             tc.tile_pool(name="ps_misc", bufs=1, space="PSUM") as ps_misc:

            from concourse.masks import make_identity
            ident = const.tile([128, 128], f32, tag="ident")
            make_identity(nc, ident[:])

            # PE warm-up (HAM 8/8 before projections) + ACT exp-table preload
            warm_src = const.tile([128, 512], f32, tag="wsrc")
            nc.vector.memset(warm_src[:], 0.001)
            warm_exp = const.tile([1, 16], f32, tag="wexp")
            nc.scalar.activation(warm_exp[:], warm_src[0:1, 0:16], Exp)
            for _ in range(16):
                wps = ps_misc.tile([1, 512], f32, tag="sacc", name="wps")
                nc.tensor.matmul(wps[:], warm_src[:, 0:1], warm_src[:],
                                 start=True, stop=True)

            bq4 = const.tile([128, 1], f32, tag="bq4")
            bk4 = const.tile([128, 1], f32, tag="bk4")
            for j in range(4):
                nc.gpsimd.dma_start(bq4[32 * j:32 * (j + 1), :], bq_d[:, None])
                nc.gpsimd.dma_start(bk4[32 * j:32 * (j + 1), :], bk_d[:, None])
            bv2 = const.tile([128, 2], f32, tag="bv2")
            nc.gpsimd.dma_start(bv2[:], bv_d.rearrange("(o p) -> p o", p=128))
            g_col = const.tile([128, 1], f32, tag="gcol")
            nc.gpsimd.dma_start(g_col[:], g_d[None, :].to_broadcast([128, 1]))
            # gamma/240 column for the tail (fp8 weights carry a 240x scale)
            g240 = const.tile([128, 1], f32, tag="g240")
            nc.vector.tensor_scalar_mul(g240[:], g_col[:], 1.0 / 240.0)

            ones1 = const.tile([128, 1], bf16, tag="ones1")
            nc.any.memset(ones1[:], 1.0)
            ones4_raw = work.tile([4, 128], f32, tag="o4raw", bufs=1)
            nc.any.memset(ones4_raw[:], 1.0)
            ones4 = const.tile([4, 128], f32r, tag="ones4")
            nc.vector.tensor_copy(ones4[:], ones4_raw[:])

            gbv = const.tile([128, 2], f32, tag="gbv")
            nc.vector.tensor_scalar_mul(gbv[:], bv2[:], g_col[:])

            wq_nat = work.tile([CK, C], f32, tag="wnat", bufs=2)
            nc.sync.dma_start(wq_nat[:], wq_d[:])
            wk_nat = work.tile([CK, C], f32, tag="wnat", bufs=2)
            nc.sync.dma_start(wk_nat[:], wk_d[:])
            wqT4 = const.tile([128, 2, 128], bf16, tag="wqT4")
            wkT4 = const.tile([128, 2, 128], bf16, tag="wkT4")
            for nat, dstw in ((wq_nat, wqT4), (wk_nat, wkT4)):
                for o in range(2):
                    tp = ps_out.tile([128, CK], f32, tag="out")
                    nc.tensor.transpose(tp[:], nat[:, ds(128 * o, 128)],
                                        ident[0:CK, 0:CK])
                    for j in range(4):
                        nc.vector.tensor_copy(dstw[:, o, ds(32 * j, 32)], tp[:])

            wv_nat = work.tile([128, 2, C], f32, tag="wvnat", bufs=1)
            wv_n = wv_d.rearrange("(o p) c -> p o c", p=128)
            for o in range(2):
                nc.sync.dma_start(wv_nat[:, o], wv_n[:, o])
            wvT = const.tile([128, 2, C], bf16, tag="wvT")
            for o_c in range(2):
                for o_co in range(2):
                    tp = ps_out.tile([128, 128], f32, tag="out")
                    nc.tensor.transpose(tp[:], wv_nat[:, o_co, ds(128 * o_c, 128)],
                                        ident[:])
                    nc.vector.tensor_copy(wvT[:, o_c, ds(128 * o_co, 128)], tp[:])

            xs = big.tile([128, 2, N], f32, tag="xs")
            xr = big.tile([128, 2, N], bf16, tag="xr")
            q4c = [big.tile([128, 512], bf16, tag=f"q4_{i}", name=f"q4_{i}")
                   for i in range(MC)]
            k4c = [big.tile([128, 512], bf16, tag=f"k4_{i}", name=f"k4_{i}")
                   for i in range(MC)]
            # v in fp8 (e4m3): direct cast of Wv@x (bias folded into residual)
            vT8 = [big.tile([128, 4, C], fp8, tag=f"v8_{i}", name=f"v8_{i}")
                   for i in range(MC)]
            for mc in range(MC):
                ms = ds(512 * mc, 512)
                nc.sync.dma_start(xs[:, :, ms], x_r[:, :, ms])

            # ---------- projections ----------
            for mc in range(MC):
                ms = ds(512 * mc, 512)
                nc.vector.tensor_copy(xr[:, :, ms], xs[:, :, ms])
                for w_t, b4, dst in ((wqT4, bq4, q4c[mc]), (wkT4, bk4, k4c[mc])):
                    pp = ps_out.tile([128, 512], f32, tag="out")
                    for o in range(2):
                        nc.tensor.matmul(pp[:], w_t[:, o, :], xr[:, o, ms],
                                         start=(o == 0), stop=(o == 1))
                    nc.scalar.activation(dst[:], pp[:], Identity, bias=b4[:])
                for nb in range(4 * mc, 4 * mc + 4):
                    pv = ps_out.tile([128, C], f32, tag="out")
                    for o in range(2):
                        nc.tensor.matmul(pv[:], xr[:, o, ds(128 * nb, 128)],
                                         wvT[:, o, :], start=(o == 0), stop=(o == 1))
                    if nb % 2 == 0:
                        nc.vector.tensor_copy(vT8[mc][:, nb - 4 * mc, :], pv[:])
                    else:
                        nc.scalar.copy(vT8[mc][:, nb - 4 * mc, :], pv[:])
                for o in range(2):
                    nc.vector.tensor_scalar_add(xs[:, o, ms], xs[:, o, ms],
                                                gbv[:, o:o + 1])

            # ---------- attention ----------
            def st_group(mc, g):
                st = ps_st.tile([128, 2048], f32, tag="st", name=f"st_{mc}_{g}")
                for j in range(4):
                    nc.tensor.matmul(st[:, ds(512 * j, 512)],
                                     k4c[g][32 * j:32 * (j + 1),
                                            ds(128 * j, 128)],
                                     q4c[mc][32 * j:32 * (j + 1), :],
                                     start=True, stop=True,
                                     tile_position=(32 * j, 0))
                pt = ptp.tile([128, 2048], bf16, tag=f"pt{g}",
                              name=f"pt_{mc}_{g}")
                nc.scalar.activation(pt[:], st[:], Exp)
                return pt

            def ssums(g, pt, s_ps):
                for j in range(4):
                    nc.tensor.matmul(s_ps[32 * j:32 * j + 1, :],
                                     ones1[:], pt[:, ds(512 * j, 512)],
                                     start=(g == 0), stop=(g == NG - 1),
                                     tile_position=(0, 32 * j))

            def make_consume(mc, pts, s4_sb):
                ms = ds(512 * mc, 512)
                out_ps = [ps_out.tile([128, 512], f32, tag="out",
                                      name=f"out_{mc}_{cc}") for cc in range(2)]
                r4 = work.tile([128, 4, 512], bf16, tag="r4", bufs=2,
                               name=f"r4_{mc}")
                pt8s = {}

                def consume(phase, g=None):
                    if phase == "head":
                        srep_ps = ps_misc.tile([128, 512], f32, tag="srep")
                        nc.tensor.matmul(srep_ps[:], ones4[:], s4_sb[:],
                                         start=True, stop=True)
                        r_rep = work.tile([128, 512], f32, tag="rrep", bufs=2,
                                          name=f"rrep_{mc}")
                        # s4_sb already carries 1/240 -> r_rep = 240/S
                        nc.vector.reciprocal_approx_fast(r_rep[:], srep_ps[:])
                        for j in range(4):
                            nc.vector.tensor_copy(r4[:, j, :], r_rep[:])
                    elif phase == "quant":
                        pt8 = pt8p.tile([128, 4, 512], fp8, tag=f"pt8_{g}",
                                        name=f"pt8_{mc}_{g}")
                        nc.vector.tensor_mul(pt8[:], pts[g][:].rearrange(
                            "p (f n) -> p f n", f=4), r4[:])
                        pt8s[g] = pt8
                    elif phase == "out":
                        for i in range(2):
                            p = 2 * g + i
                            for cc in range(2):
                                nc.tensor.matmul(
                                    out_ps[cc][:],
                                    vT8[g][:, 2 * i:2 * i + 2,
                                           ds(128 * cc, 128)],
                                    pt8s[g][:, 2 * i:2 * i + 2, :],
                                    start=(p == 0), stop=(p == 15),
                                    perf_mode=DR)
                    elif phase == "tail":
                        for cc in range(2):
                            y_sb = work.tile([128, 512], f32, tag="y", bufs=2,
                                             name=f"y_{mc}_{cc}")
                            nc.vector.scalar_tensor_tensor(
                                y_sb[:], out_ps[cc][:], g240[:],
                                xs[:, cc, ms], Mult, Add)
                            eng = nc.sync if cc == 0 else nc.gpsimd
                            eng.dma_start(y_r[:, cc, ms], y_sb[:])
                return consume

            consume = None
            for mc in range(MC):
                s_ps = ps_misc.tile([128, 512], f32, tag="sacc",
                                    name=f"sacc_{mc}")
                pts = {}
                for g in range(NG):
                    pts[g] = st_group(mc, g)
                    if g == 0 and consume is not None:
                        consume("head")
                    if g > 0:
                        ssums(g - 1, pts[g - 1], s_ps)
                    if consume is not None:
                        consume("quant", g)
                        consume("out", g)
                ssums(NG - 1, pts[NG - 1], s_ps)
                if consume is not None:
                    consume("tail")
                # denominator partials -> SBUF with 1/240 folded in
                s4c = work.tile([128, 512], f32r, tag="s4c", bufs=2,
                                name=f"s4c_{mc}")
                nc.vector.tensor_scalar_mul(s4c[:], s_ps[:], 1.0 / 240.0)
                s4_sb = work.tile([4, 512], f32r, tag="s4", bufs=2,
                                  name=f"s4_{mc}")
                nc.gpsimd.dma_start(s4_sb[:], s4c[0:97:32, :])
                consume = make_consume(mc, pts, s4_sb)

            # drain last chunk
            consume("head")
            for g in range(NG):
                consume("quant", g)
                consume("out", g)
            consume("tail")

    nc.compile()
    return nc


def kernel(x, Wq, bq, Wk, bk, Wv, bv, gamma):
    from concourse import bass_utils

    if "nc" not in _NC_CACHE:
        _NC_CACHE["nc"] = _build_nc()
    nc = _NC_CACHE["nc"]

    x = np.ascontiguousarray(np.asarray(x, dtype=np.float32))
    shared = {
        "Wq": np.ascontiguousarray(np.asarray(Wq, dtype=np.float32)),
        "bq": np.ascontiguousarray(np.asarray(bq, dtype=np.float32)),
        "Wk": np.ascontiguousarray(np.asarray(Wk, dtype=np.float32)),
        "bk": np.ascontiguousarray(np.asarray(bk, dtype=np.float32)),
        "Wv": np.ascontiguousarray(np.asarray(Wv, dtype=np.float32)),
        "bv": np.ascontiguousarray(np.asarray(bv, dtype=np.float32)),
        "gamma": np.ascontiguousarray(np.asarray(gamma, dtype=np.float32)),
    }
    in_maps = [dict(shared, x=np.ascontiguousarray(x[i].reshape(C, N)))
               for i in range(B)]

    res = bass_utils.run_bass_kernel_spmd(nc, in_maps, core_ids=list(range(B)))
    y = np.stack([res.results[i]["y"] for i in range(B)], axis=0)
    return y.reshape(B, C, H, W).astype(np.float32)
